# revision 68
# baseline (speedup 1.0000x reference)
"""GenAttentionAggregation Trainium2 kernel.

Computes, for N=131072 nodes, D=512, SEG=4096 segments:
  h = x @ W_emb + b_emb
  scores = (attention_x @ W_score + b_score)[:, 0]
  weights = segment_softmax(scores, index, SEG)
  pooled = segment_sum(h * weights[:, None], index, SEG)
  out = pooled * (counts @ W_size + b_size)

Device strategy (all heavy compute on the 8 NeuronCores):
  The aggregation is reordered before the embedding matmul:
    pooled[s] = (sum_{i in s} e_i * x_i) @ W_emb / denom[s] + b_emb
  with e_i = exp(score_i), denom[s] = sum_{i in s} e_i.  This shrinks the
  dominant matmul from [N,D]x[D,D] to [SEG,D]x[D,D] and turns the segment
  scatter into small one-hot matmuls.

  Sharding: nodes are sorted by segment on the host (index-only metadata);
  core c owns the aligned segment range [512c, 512c+512) and receives
  exactly the nodes landing there, so no cross-core reduction is needed.
  Each core processes 4 "windows" of 128 segments; within a window, nodes
  are packed into tiles of 128 with a host-built one-hot matrix
  A[node, local_seg].  Per tile the device computes
    scores = rowsum(ax_tile * W_score_rep)        (DVE, fused mul+reduce)
    e = exp(scores + b_score)                     (ACT)
    Ae = A_tile * e                               (DVE, per-partition scalar)
    S_psum  += Ae^T @ x_tile                      (PE, PSUM accumulate)
    den_psum += Ae^T @ ones                       (PE)
  and per window:
    out_w = ((S^T)^T @ W_emb) / den + b_emb, scaled by counts*W_size+b_size.

All inputs are cast to bf16 for the matmul/score paths (fp32 accumulation
everywhere); validated end-to-end rel error ~4e-3 vs the fp32 reference.
"""

import os
import numpy as np

N = 131072
D = 512
SEG = 4096
EPS = 1e-16
NCORES = 8
P = 128
SEGC = SEG // NCORES          # 512 segments per core
WINS = SEGC // P              # 4 windows of 128 segments per core

LAST_EXEC_NS = None
_DEVICE_OK = None
_BUILD_CACHE = {}


def _build_device_program(TW):
    """Compile the per-core Bass program for TW tiles (of 128 nodes) per
    window.  All 8 cores run the same program on different data."""
    import sys
    for p in ("/opt/trn_rl_repo",):
        if p not in sys.path:
            sys.path.insert(0, p)
    from contextlib import ExitStack
    from concourse import bacc, tile, mybir
    from concourse.tile import add_dep_helper

    bf16 = mybir.dt.bfloat16
    f32 = mybir.dt.float32
    mult = mybir.AluOpType.mult
    add = mybir.AluOpType.add
    is_equal = mybir.AluOpType.is_equal
    Exp = mybir.ActivationFunctionType.Exp

    nc = bacc.Bacc("TRN2", target_bir_lowering=False)

    NN = TW * P                    # nodes per window
    GPW = -(-TW // 4)              # groups (of <=4 tiles / 512 nodes) per window
    # Packed per-group input block: slot k holds attention_x D-chunk k
    # (transposed, node-major) in [:, k, 0:512] and x tile t0+k in
    # [:, k, 512:1024], so each group is ONE contiguous ~2 MB DMA.
    grp_t = nc.dram_tensor("grp", (WINS, GPW, P, 4, 1024), bf16,
                           kind="ExternalInput")
    lidx_t = nc.dram_tensor("lidx", (WINS, P, TW), f32, kind="ExternalInput")
    iota_t = nc.dram_tensor("iota", (P, P), bf16, kind="ExternalInput")
    wemb_t = nc.dram_tensor("wemb", (P, 4, D), bf16, kind="ExternalInput")
    wsc4_t = nc.dram_tensor("wsc4", (P, 4), bf16, kind="ExternalInput")
    cnts_t = nc.dram_tensor("cnts", (P, WINS), f32, kind="ExternalInput")
    wszr_t = nc.dram_tensor("wszr", (P, D), f32, kind="ExternalInput")
    bszr_t = nc.dram_tensor("bszr", (P, D), f32, kind="ExternalInput")
    bembr_t = nc.dram_tensor("bembr", (P, D), f32, kind="ExternalInput")
    bscr_t = nc.dram_tensor("bscr", (P, 1), f32, kind="ExternalInput")
    ones_t = nc.dram_tensor("ones", (P, 1), bf16, kind="ExternalInput")
    onesf_t = nc.dram_tensor("onesf", (P, 1), f32, kind="ExternalInput")
    ident_t = nc.dram_tensor("ident", (P, P), bf16, kind="ExternalInput")
    out_t = nc.dram_tensor("out", (WINS, P, D), f32, kind="ExternalOutput")

    with tile.TileContext(nc) as tc, ExitStack() as ctx:
        const_p = ctx.enter_context(tc.tile_pool(name="const", bufs=1))
        win_p = ctx.enter_context(tc.tile_pool(name="win", bufs=2))
        grp_p = ctx.enter_context(tc.tile_pool(name="grp", bufs=10))
        work_p = ctx.enter_context(tc.tile_pool(name="work", bufs=8))
        big_p = ctx.enter_context(tc.tile_pool(name="big", bufs=2))
        ps_s = ctx.enter_context(tc.tile_pool(name="psS", bufs=2, space="PSUM"))
        ps_d = ctx.enter_context(tc.tile_pool(name="psD", bufs=1, space="PSUM"))
        ps_c = ctx.enter_context(tc.tile_pool(name="psC", bufs=3, space="PSUM"))
        ps_e = ctx.enter_context(tc.tile_pool(name="psE", bufs=1, space="PSUM"))
        ps_o = ctx.enter_context(tc.tile_pool(name="psO", bufs=1, space="PSUM"))

        wemb_sb = const_p.tile([P, 4, D], bf16)
        nc.sync.dma_start(out=wemb_sb[:], in_=wemb_t[:])
        wsc4_sb = const_p.tile([P, 4], bf16)
        nc.sync.dma_start(out=wsc4_sb[:], in_=wsc4_t[:])
        cnts_sb = const_p.tile([P, WINS], f32)
        nc.sync.dma_start(out=cnts_sb[:], in_=cnts_t[:])
        wszr_sb = const_p.tile([P, D], f32)
        nc.sync.dma_start(out=wszr_sb[:], in_=wszr_t[:])
        bszr_sb = const_p.tile([P, D], f32)
        nc.sync.dma_start(out=bszr_sb[:], in_=bszr_t[:])
        bembr_sb = const_p.tile([P, D], f32)
        nc.sync.dma_start(out=bembr_sb[:], in_=bembr_t[:])
        bscr_sb = const_p.tile([P, 1], f32)
        nc.sync.dma_start(out=bscr_sb[:], in_=bscr_t[:])
        ones_sb = const_p.tile([P, 1], bf16)
        nc.sync.dma_start(out=ones_sb[:], in_=ones_t[:])
        onesf_sb = const_p.tile([P, 1], f32)
        nc.sync.dma_start(out=onesf_sb[:], in_=onesf_t[:])
        ident_sb = const_p.tile([P, P], bf16)
        nc.sync.dma_start(out=ident_sb[:], in_=ident_t[:])
        iota_sb = const_p.tile([P, P], bf16)
        nc.sync.dma_start(out=iota_sb[:], in_=iota_t[:])

        # Node ordering within a group is t-major (j = t'*128 + p).  Work is
        # emitted as a software pipeline over groups: the load+score stage
        # for group g+LA is emitted before the aggregation stage of group g,
        # so the PE (strict FIFO) never head-of-line blocks on the
        # exp/reshape round trip, and compute starts after the first ~2 MB
        # chunk instead of after a whole 9 MB window.
        LA = 5                         # pipeline lookahead (groups)
        groups = [(w, gw) for w in range(WINS) for gw in range(GPW)]
        state = {}

        def load_stage(gi):
            w, gw = groups[gi]
            t0g = gw * 4
            t1g = min(t0g + 4, TW)
            nt = t1g - t0g
            if gw == 0:
                lx = win_p.tile([P, TW], f32, tag="lx")
                nc.sync.dma_start(out=lx[:], in_=lidx_t[w])
                state[("w", w)] = lx
            gq = grp_p.tile([P, 4, 1024], bf16, tag="gq")
            nc.sync.dma_start(out=gq[:], in_=grp_t[w, gw])
            psC = ps_c.tile([1, 512], f32, tag="psC")
            for c in range(4):
                sm = nc.tensor.matmul(
                    out=psC[:, :nt * P], lhsT=wsc4_sb[:, c:c + 1],
                    rhs=gq[:, c, :nt * P], start=(c == 0), stop=(c == 3),
                )
                if c == 0:
                    state[("Sfirst", gi)] = sm
            state[("Slast", gi)] = sm
            e_chunk = work_p.tile([1, 512], f32, tag="e_chunk")
            nc.scalar.activation(
                out=e_chunk[:, :nt * P], in_=psC[:, :nt * P], func=Exp,
                bias=bscr_sb[:1, :1], scale=1.0,
            )
            state[gi] = (gq, e_chunk, t0g, t1g)

        def body_stage(gi):
            w, gw = groups[gi]
            lx = state[("w", w)]
            gq, e_chunk, t0g, t1g = state.pop(gi)
            if gw == 0:
                state[("ps", w)] = (
                    ps_s.tile([P, D], f32, tag="psS", name="psS"),
                    ps_d.tile([P, 1], f32, tag="psD", name="psD"),
                )
            psS, psD = state[("ps", w)]
            # e columns for this group's tiles: [1, 128] -> [128, 1] via PE
            # transpose (K=1 matmul against ones), used straight from PSUM.
            # Deterministic PE order [B(i), T(i+2), S(i+3)]: every
            # cross-engine latency (exp -> transpose, Ae -> aggregate) is
            # covered by at least one group-cycle of PE work.
            psE = ps_e.tile([P, 4], f32, tag="psE", name="psE")
            for t in range(t0g, t1g):
                k = t - t0g
                tr = nc.tensor.transpose(
                    out=psE[:, k:k + 1], in_=e_chunk[:, k * P:(k + 1) * P],
                    identity=onesf_sb[:1, :1],
                )
                if k == 0 and ("Slast", gi + 2) in state:
                    add_dep_helper(tr.ins, state[("Slast", gi + 2)].ins,
                                   sync=False, reason="pe-order: T(i) after S(i+2)")

            for t in range(t0g, t1g):
                k = t - t0g
                ae = work_p.tile([P, P], bf16, tag="ae")
                nc.vector.tensor_scalar(
                    out=ae[:], in0=iota_sb[:], scalar1=lx[:, t:t + 1],
                    scalar2=psE[:, k:k + 1], op0=is_equal, op1=mult,
                )
                bm = nc.tensor.matmul(
                    out=psS[:], lhsT=ae[:], rhs=gq[:, k, 512:],
                    start=(t == 0), stop=(t == TW - 1),
                )

                bm = nc.tensor.matmul(
                    out=psD[:], lhsT=ae[:], rhs=ones_sb[:],
                    start=(t == 0), stop=(t == TW - 1),
                )
            state[("Blast", gi)] = bm
            for key in (("Blast", gi - 3), ("Sfirst", gi - 2), ("Slast", gi - 2)):
                state.pop(key, None)

        def epilogue(w):
            psS, psD = state.pop(("ps", w))
            # Window epilogue: out_w = (S @ W_emb)/den + b_emb, scaled.
            s_sb = work_p.tile([P, D], bf16, tag="s_sb")
            nc.scalar.copy(out=s_sb[:], in_=psS[:])
            d_sb = work_p.tile([P, 1], f32, tag="d_sb")
            nc.vector.tensor_scalar(
                out=d_sb[:], in0=psD[:], scalar1=float(EPS), scalar2=None, op0=add,
            )
            rd = work_p.tile([P, 1], f32, tag="rd")
            nc.vector.reciprocal(out=rd[:], in_=d_sb[:])

            psO = ps_o.tile([P, D], f32, tag="psO")
            for c in range(4):
                psT = ps_e.tile([P, P], bf16, tag="psE", name="psT")
                nc.tensor.transpose(
                    out=psT[:], in_=s_sb[:, c * P:(c + 1) * P], identity=ident_sb[:],
                )
                st = work_p.tile([P, P], bf16, tag="st")
                nc.scalar.copy(out=st[:], in_=psT[:])
                nc.tensor.matmul(
                    out=psO[:], lhsT=st[:], rhs=wemb_sb[:, c],
                    start=(c == 0), stop=(c == 3),
                )

            us = big_p.tile([P, D], f32, tag="us")
            nc.vector.scalar_tensor_tensor(
                out=us[:], in0=wszr_sb[:], scalar=cnts_sb[:, w:w + 1],
                in1=bszr_sb[:], op0=mult, op1=add,
            )
            pw = big_p.tile([P, D], f32, tag="pw")
            nc.vector.scalar_tensor_tensor(
                out=pw[:], in0=psO[:], scalar=rd[:, :1],
                in1=bembr_sb[:], op0=mult, op1=add,
            )
            fin = big_p.tile([P, D], f32, tag="fin")
            nc.vector.tensor_tensor(out=fin[:], in0=pw[:], in1=us[:], op=mult)
            nc.scalar.dma_start(out=out_t[w], in_=fin[:])

        for gi in range(len(groups) + LA):
            if gi < len(groups):
                load_stage(gi)
            if gi >= LA:
                body_stage(gi - LA)
                bw, bgw = groups[gi - LA]
                if bgw == GPW - 1:
                    epilogue(bw)

    nc.compile()
    return nc


def _ensure_ntff_hook():
    """Provide the antenv.axon_hooks shim the boot script expects, so
    run_bass_kernel_spmd(trace=True) can capture NTFF profiles under axon."""
    import sys
    import types
    try:
        from antenv.axon_hooks import get_axon_ntff_profile_hook
        if get_axon_ntff_profile_hook() is not None:
            return True
    except ImportError:
        pass
    try:
        import antenv
        mod = types.ModuleType("antenv.axon_hooks")
        mod._hook = None

        def set_axon_ntff_profile_hook(h):
            mod._hook = h

        def get_axon_ntff_profile_hook():
            return mod._hook

        mod.set_axon_ntff_profile_hook = set_axon_ntff_profile_hook
        mod.get_axon_ntff_profile_hook = get_axon_ntff_profile_hook
        sys.modules["antenv.axon_hooks"] = mod
        antenv.axon_hooks = mod
        from trn_agent_boot.trn_boot import _ntff_profile_via_ctypes
        hook = _ntff_profile_via_ctypes("/opt/axon/libaxon_pjrt.so")
        if hook is None:
            return False
        set_axon_ntff_profile_hook(hook)
        return True
    except Exception:
        return False


def _device_run(x, ax, W_emb, b_emb, W_score, b_score, W_size, b_size, idx):
    global LAST_EXEC_NS
    import sys
    for p in ("/opt/trn_rl_repo",):
        if p not in sys.path:
            sys.path.insert(0, p)
    import ml_dtypes
    from concourse.bass_utils import run_bass_kernel_spmd

    bf = ml_dtypes.bfloat16

    counts = np.bincount(idx, minlength=SEG)[:SEG].astype(np.int64)
    order = np.argsort(idx, kind="stable")
    NW = SEG // P                                   # 32 global windows
    wcnt = counts.reshape(NW, P).sum(1)
    TW = max(1, int(-(-wcnt.max() // P)))           # tiles per window

    nc = _BUILD_CACHE.get(TW)
    if nc is None:
        nc = _build_device_program(TW)
        _BUILD_CACHE[TW] = nc

    # Gather map g[c, w, p, t] = node id; pad slots point at row 0 and get a
    # zero one-hot row so they contribute nothing.
    g = np.zeros((NW, TW * P), np.int64)
    valid = np.zeros((NW, TW * P), bool)
    wstart = np.concatenate([[0], np.cumsum(wcnt)])
    for gw in range(NW):
        nodes = order[wstart[gw]:wstart[gw + 1]]
        g[gw, :len(nodes)] = nodes
        valid[gw, :len(nodes)] = True
    # tile-major [TW, P] -> [P, TW]
    g = g.reshape(NCORES, WINS, TW, P).transpose(0, 1, 3, 2)
    valid = valid.reshape(NCORES, WINS, TW, P).transpose(0, 1, 3, 2)

    lseg = (idx[g] - (np.arange(NW).reshape(NCORES, WINS, 1, 1) * P)).astype(np.int64)
    lidx = np.where(valid, lseg, -1).astype(np.float32)
    iota = np.broadcast_to(np.arange(P, dtype=np.float32)[None, :], (P, P)).astype(bf)

    x_bf = x.astype(bf, copy=False)
    ax_bf = ax.astype(bf, copy=False)
    GPW = -(-TW // 4)
    TWp = GPW * 4
    # Pad the gather map to a multiple of 4 tiles; pad tiles point at row 0
    # with lidx=-1 so they contribute nothing.
    gp = np.zeros((NCORES, WINS, P, TWp), np.int64)
    gp[..., :TW] = g
    xg = x_bf[gp]                                   # [NCORES, WINS, P, TWp, D]
    axg = ax_bf[gp]
    # Packed per-group block [c, w, gw, dp/p, slot, 1024]:
    #   [:, k, 0:512]    = attention_x D-chunk k, transposed, node-major
    #   [:, k, 512:1024] = x rows of tile t0+k
    ax_blk = (axg.transpose(0, 1, 3, 2, 4)          # [c, w, t, p, D]
              .reshape(NCORES, WINS, GPW, 4 * P, 4, P)
              .transpose(0, 1, 2, 5, 4, 3))         # [c, w, gw, dp, dc, 512]
    x_blk = (xg.reshape(NCORES, WINS, P, GPW, 4, D)
             .transpose(0, 1, 3, 2, 4, 5))          # [c, w, gw, p, slot, D]
    grp_pack = np.concatenate(
        [np.ascontiguousarray(ax_blk), np.ascontiguousarray(x_blk)], axis=-1)

    wemb = np.ascontiguousarray(
        W_emb.astype(bf).reshape(4, P, D).transpose(1, 0, 2))
    wsc4 = np.ascontiguousarray(W_score.astype(bf)[:, 0].reshape(4, P).T)
    cnts = np.ascontiguousarray(
        counts.astype(np.float32).reshape(NCORES, WINS, P).transpose(0, 2, 1))
    wszr = np.broadcast_to(W_size.astype(np.float32)[0], (P, D)).copy()
    bszr = np.broadcast_to(b_size.astype(np.float32), (P, D)).copy()
    bembr = np.broadcast_to(b_emb.astype(np.float32), (P, D)).copy()
    bscr = np.full((P, 1), np.float32(b_score[0]), np.float32)
    ones = np.ones((P, 1), bf)
    onesf = np.ones((P, 1), np.float32)
    ident = np.eye(P, dtype=bf)

    in_maps = []
    for c in range(NCORES):
        in_maps.append({
            "grp": grp_pack[c],
            "lidx": np.ascontiguousarray(lidx[c]),
            "iota": np.ascontiguousarray(iota),
            "wemb": wemb, "wsc4": wsc4, "cnts": cnts[c],
            "wszr": wszr, "bszr": bszr, "bembr": bembr,
            "bscr": bscr, "ones": ones, "onesf": onesf, "ident": ident,
        })

    trace = bool(os.environ.get("BASS_KERNEL_TRACE"))
    if trace:
        trace = _ensure_ntff_hook()
    res = run_bass_kernel_spmd(nc, in_maps, core_ids=list(range(NCORES)),
                               trace=trace)
    outs = res.results if hasattr(res, "results") else res
    if hasattr(res, "exec_time_ns") and res.exec_time_ns is not None:
        LAST_EXEC_NS = res.exec_time_ns
    out = np.concatenate(
        [np.asarray(o["out"], np.float32).reshape(SEGC, D) for o in outs], axis=0)
    return out


def _host_run(x, ax, W_emb, b_emb, W_score, b_score, W_size, b_size, idx):
    """Numpy fallback (only used if the device toolchain is unavailable)."""
    h = x @ W_emb + b_emb
    scores = (ax @ W_score)[:, 0] + b_score[0]
    order = np.argsort(idx, kind="stable")
    idx_s = idx[order]
    counts = np.bincount(idx_s, minlength=SEG)[:SEG]
    starts = np.zeros(SEG, np.int64)
    np.cumsum(counts[:-1], out=starts[1:])
    starts_c = np.minimum(starts, max(len(idx_s) - 1, 0))
    nonempty = counts > 0
    scores_s = scores[order]
    seg_max = np.maximum.reduceat(scores_s, starts_c)
    wgt = np.exp(scores - seg_max[idx])
    denom = np.add.reduceat(wgt[order], starts_c)
    weights = wgt / (denom[idx] + EPS)
    weighted = (h * weights[:, None]).astype(np.float32)
    pooled = np.add.reduceat(weighted[order], starts_c, axis=0)
    pooled[~nonempty] = 0.0
    updated = counts.astype(np.float32)[:, None] @ W_size + b_size
    return (pooled * updated).astype(np.float32)


def kernel(x, attention_x, W_emb, b_emb, W_score, b_score, W_size, b_size,
           index, size):
    global _DEVICE_OK
    x = np.asarray(x, dtype=np.float32)
    attention_x = np.asarray(attention_x, dtype=np.float32)
    W_emb = np.asarray(W_emb, dtype=np.float32)
    b_emb = np.asarray(b_emb, dtype=np.float32)
    W_score = np.asarray(W_score, dtype=np.float32)
    b_score = np.asarray(b_score, dtype=np.float32)
    W_size = np.asarray(W_size, dtype=np.float32)
    b_size = np.asarray(b_size, dtype=np.float32)
    idx = np.asarray(index).astype(np.int64).ravel()

    try:
        out = _device_run(x, attention_x, W_emb, b_emb, W_score, b_score,
                          W_size, b_size, idx)
        _DEVICE_OK = True
        return out
    except Exception:
        if os.environ.get("BASS_KERNEL_RAISE"):
            raise
        _DEVICE_OK = False
        return _host_run(x, attention_x, W_emb, b_emb, W_score, b_score,
                         W_size, b_size, idx)


# revision 69
# speedup vs baseline: 1.0020x; 1.0020x over previous
"""GenAttentionAggregation Trainium2 kernel.

Computes, for N=131072 nodes, D=512, SEG=4096 segments:
  h = x @ W_emb + b_emb
  scores = (attention_x @ W_score + b_score)[:, 0]
  weights = segment_softmax(scores, index, SEG)
  pooled = segment_sum(h * weights[:, None], index, SEG)
  out = pooled * (counts @ W_size + b_size)

Device strategy (all heavy compute on the 8 NeuronCores):
  The aggregation is reordered before the embedding matmul:
    pooled[s] = (sum_{i in s} e_i * x_i) @ W_emb / denom[s] + b_emb
  with e_i = exp(score_i), denom[s] = sum_{i in s} e_i.  This shrinks the
  dominant matmul from [N,D]x[D,D] to [SEG,D]x[D,D] and turns the segment
  scatter into small one-hot matmuls.

  Sharding: nodes are sorted by segment on the host (index-only metadata);
  core c owns the aligned segment range [512c, 512c+512) and receives
  exactly the nodes landing there, so no cross-core reduction is needed.
  Each core processes 4 "windows" of 128 segments; within a window, nodes
  are packed into tiles of 128 with a host-built one-hot matrix
  A[node, local_seg].  Per tile the device computes
    scores = rowsum(ax_tile * W_score_rep)        (DVE, fused mul+reduce)
    e = exp(scores + b_score)                     (ACT)
    Ae = A_tile * e                               (DVE, per-partition scalar)
    S_psum  += Ae^T @ x_tile                      (PE, PSUM accumulate)
    den_psum += Ae^T @ ones                       (PE)
  and per window:
    out_w = ((S^T)^T @ W_emb) / den + b_emb, scaled by counts*W_size+b_size.

All inputs are cast to bf16 for the matmul/score paths (fp32 accumulation
everywhere); validated end-to-end rel error ~4e-3 vs the fp32 reference.
"""

import os
import numpy as np

N = 131072
D = 512
SEG = 4096
EPS = 1e-16
NCORES = 8
P = 128
SEGC = SEG // NCORES          # 512 segments per core
WINS = SEGC // P              # 4 windows of 128 segments per core

LAST_EXEC_NS = None
_DEVICE_OK = None
_BUILD_CACHE = {}


def _build_device_program(TW):
    """Compile the per-core Bass program for TW tiles (of 128 nodes) per
    window.  All 8 cores run the same program on different data."""
    import sys
    for p in ("/opt/trn_rl_repo",):
        if p not in sys.path:
            sys.path.insert(0, p)
    from contextlib import ExitStack
    from concourse import bacc, tile, mybir
    from concourse.tile import add_dep_helper

    bf16 = mybir.dt.bfloat16
    f32 = mybir.dt.float32
    mult = mybir.AluOpType.mult
    add = mybir.AluOpType.add
    is_equal = mybir.AluOpType.is_equal
    Exp = mybir.ActivationFunctionType.Exp

    nc = bacc.Bacc("TRN2", target_bir_lowering=False)

    NN = TW * P                    # nodes per window
    GPW = -(-TW // 4)              # groups (of <=4 tiles / 512 nodes) per window
    # Packed per-group input block: slot k holds attention_x D-chunk k
    # (transposed, node-major) in [:, k, 0:512] and x tile t0+k in
    # [:, k, 512:1024], so each group is ONE contiguous ~2 MB DMA.
    grp_t = nc.dram_tensor("grp", (WINS, GPW, P, 4, 1024), bf16,
                           kind="ExternalInput")
    lidx_t = nc.dram_tensor("lidx", (WINS, P, TW), f32, kind="ExternalInput")
    iota_t = nc.dram_tensor("iota", (P, P), bf16, kind="ExternalInput")
    wemb_t = nc.dram_tensor("wemb", (P, 4, D), bf16, kind="ExternalInput")
    wsc4_t = nc.dram_tensor("wsc4", (P, 4), bf16, kind="ExternalInput")
    cnts_t = nc.dram_tensor("cnts", (P, WINS), f32, kind="ExternalInput")
    wszr_t = nc.dram_tensor("wszr", (P, D), f32, kind="ExternalInput")
    bszr_t = nc.dram_tensor("bszr", (P, D), f32, kind="ExternalInput")
    bembr_t = nc.dram_tensor("bembr", (P, D), f32, kind="ExternalInput")
    bscr_t = nc.dram_tensor("bscr", (P, 1), f32, kind="ExternalInput")
    ones_t = nc.dram_tensor("ones", (P, 1), bf16, kind="ExternalInput")
    onesf_t = nc.dram_tensor("onesf", (P, 1), f32, kind="ExternalInput")
    ident_t = nc.dram_tensor("ident", (P, P), bf16, kind="ExternalInput")
    out_t = nc.dram_tensor("out", (WINS, P, D), f32, kind="ExternalOutput")

    with tile.TileContext(nc) as tc, ExitStack() as ctx:
        const_p = ctx.enter_context(tc.tile_pool(name="const", bufs=1))
        win_p = ctx.enter_context(tc.tile_pool(name="win", bufs=2))
        grp_p = ctx.enter_context(tc.tile_pool(name="grp", bufs=12))
        work_p = ctx.enter_context(tc.tile_pool(name="work", bufs=8))
        big_p = ctx.enter_context(tc.tile_pool(name="big", bufs=2))
        ps_s = ctx.enter_context(tc.tile_pool(name="psS", bufs=2, space="PSUM"))
        ps_d = ctx.enter_context(tc.tile_pool(name="psD", bufs=1, space="PSUM"))
        ps_c = ctx.enter_context(tc.tile_pool(name="psC", bufs=3, space="PSUM"))
        ps_e = ctx.enter_context(tc.tile_pool(name="psE", bufs=1, space="PSUM"))
        ps_o = ctx.enter_context(tc.tile_pool(name="psO", bufs=1, space="PSUM"))

        wemb_sb = const_p.tile([P, 4, D], bf16)
        nc.sync.dma_start(out=wemb_sb[:], in_=wemb_t[:])
        wsc4_sb = const_p.tile([P, 4], bf16)
        nc.sync.dma_start(out=wsc4_sb[:], in_=wsc4_t[:])
        cnts_sb = const_p.tile([P, WINS], f32)
        nc.sync.dma_start(out=cnts_sb[:], in_=cnts_t[:])
        wszr_sb = const_p.tile([P, D], f32)
        nc.sync.dma_start(out=wszr_sb[:], in_=wszr_t[:])
        bszr_sb = const_p.tile([P, D], f32)
        nc.sync.dma_start(out=bszr_sb[:], in_=bszr_t[:])
        bembr_sb = const_p.tile([P, D], f32)
        nc.sync.dma_start(out=bembr_sb[:], in_=bembr_t[:])
        bscr_sb = const_p.tile([P, 1], f32)
        nc.sync.dma_start(out=bscr_sb[:], in_=bscr_t[:])
        ones_sb = const_p.tile([P, 1], bf16)
        nc.sync.dma_start(out=ones_sb[:], in_=ones_t[:])
        onesf_sb = const_p.tile([P, 1], f32)
        nc.sync.dma_start(out=onesf_sb[:], in_=onesf_t[:])
        ident_sb = const_p.tile([P, P], bf16)
        nc.sync.dma_start(out=ident_sb[:], in_=ident_t[:])
        iota_sb = const_p.tile([P, P], bf16)
        nc.sync.dma_start(out=iota_sb[:], in_=iota_t[:])

        # Node ordering within a group is t-major (j = t'*128 + p).  Work is
        # emitted as a software pipeline over groups: the load+score stage
        # for group g+LA is emitted before the aggregation stage of group g,
        # so the PE (strict FIFO) never head-of-line blocks on the
        # exp/reshape round trip, and compute starts after the first ~2 MB
        # chunk instead of after a whole 9 MB window.
        LA = 5                         # pipeline lookahead (groups)
        groups = [(w, gw) for w in range(WINS) for gw in range(GPW)]
        state = {}

        def load_stage(gi):
            w, gw = groups[gi]
            t0g = gw * 4
            t1g = min(t0g + 4, TW)
            nt = t1g - t0g
            if gw == 0:
                lx = win_p.tile([P, TW], f32, tag="lx")
                nc.sync.dma_start(out=lx[:], in_=lidx_t[w])
                state[("w", w)] = lx
            gq = grp_p.tile([P, 4, 1024], bf16, tag="gq")
            nc.sync.dma_start(out=gq[:], in_=grp_t[w, gw])
            psC = ps_c.tile([1, 512], f32, tag="psC")
            for c in range(4):
                sm = nc.tensor.matmul(
                    out=psC[:, :nt * P], lhsT=wsc4_sb[:, c:c + 1],
                    rhs=gq[:, c, :nt * P], start=(c == 0), stop=(c == 3),
                )
                if c == 0:
                    state[("Sfirst", gi)] = sm
            state[("Slast", gi)] = sm
            e_chunk = work_p.tile([1, 512], f32, tag="e_chunk")
            nc.scalar.activation(
                out=e_chunk[:, :nt * P], in_=psC[:, :nt * P], func=Exp,
                bias=bscr_sb[:1, :1], scale=1.0,
            )
            state[gi] = (gq, e_chunk, t0g, t1g)

        def body_stage(gi):
            w, gw = groups[gi]
            lx = state[("w", w)]
            gq, e_chunk, t0g, t1g = state.pop(gi)
            if gw == 0:
                state[("ps", w)] = (
                    ps_s.tile([P, D], f32, tag="psS", name="psS"),
                    ps_d.tile([P, 1], f32, tag="psD", name="psD"),
                )
            psS, psD = state[("ps", w)]
            # e columns for this group's tiles: [1, 128] -> [128, 1] via PE
            # transpose (K=1 matmul against ones), used straight from PSUM.
            # Deterministic PE order [B(i), T(i+2), S(i+3)]: every
            # cross-engine latency (exp -> transpose, Ae -> aggregate) is
            # covered by at least one group-cycle of PE work.
            psE = ps_e.tile([P, 4], f32, tag="psE", name="psE")
            for t in range(t0g, t1g):
                k = t - t0g
                tr = nc.tensor.transpose(
                    out=psE[:, k:k + 1], in_=e_chunk[:, k * P:(k + 1) * P],
                    identity=onesf_sb[:1, :1],
                )
                if k == 0 and ("Slast", gi + 2) in state:
                    add_dep_helper(tr.ins, state[("Slast", gi + 2)].ins,
                                   sync=False, reason="pe-order: T(i) after S(i+2)")

            for t in range(t0g, t1g):
                k = t - t0g
                ae = work_p.tile([P, P], bf16, tag="ae")
                nc.vector.tensor_scalar(
                    out=ae[:], in0=iota_sb[:], scalar1=lx[:, t:t + 1],
                    scalar2=psE[:, k:k + 1], op0=is_equal, op1=mult,
                )
                bm = nc.tensor.matmul(
                    out=psS[:], lhsT=ae[:], rhs=gq[:, k, 512:],
                    start=(t == 0), stop=(t == TW - 1),
                )
                if k == 0 and ("Slast", gi + 2) in state:
                    add_dep_helper(bm.ins, state[("Slast", gi + 2)].ins,
                                   sync=False, reason="pe-order: B(i) after S(i+2)")

                bm = nc.tensor.matmul(
                    out=psD[:], lhsT=ae[:], rhs=ones_sb[:],
                    start=(t == 0), stop=(t == TW - 1),
                )
            state[("Blast", gi)] = bm
            for key in (("Blast", gi - 3), ("Sfirst", gi - 2), ("Slast", gi - 2)):
                state.pop(key, None)

        def epilogue(w):
            psS, psD = state.pop(("ps", w))
            # Window epilogue: out_w = (S @ W_emb)/den + b_emb, scaled.
            s_sb = work_p.tile([P, D], bf16, tag="s_sb")
            nc.scalar.copy(out=s_sb[:], in_=psS[:])
            d_sb = work_p.tile([P, 1], f32, tag="d_sb")
            nc.vector.tensor_scalar(
                out=d_sb[:], in0=psD[:], scalar1=float(EPS), scalar2=None, op0=add,
            )
            rd = work_p.tile([P, 1], f32, tag="rd")
            nc.vector.reciprocal(out=rd[:], in_=d_sb[:])

            psO = ps_o.tile([P, D], f32, tag="psO")
            for c in range(4):
                psT = ps_e.tile([P, P], bf16, tag="psE", name="psT")
                nc.tensor.transpose(
                    out=psT[:], in_=s_sb[:, c * P:(c + 1) * P], identity=ident_sb[:],
                )
                st = work_p.tile([P, P], bf16, tag="st")
                nc.scalar.copy(out=st[:], in_=psT[:])
                nc.tensor.matmul(
                    out=psO[:], lhsT=st[:], rhs=wemb_sb[:, c],
                    start=(c == 0), stop=(c == 3),
                )

            us = big_p.tile([P, D], f32, tag="us")
            nc.vector.scalar_tensor_tensor(
                out=us[:], in0=wszr_sb[:], scalar=cnts_sb[:, w:w + 1],
                in1=bszr_sb[:], op0=mult, op1=add,
            )
            pw = big_p.tile([P, D], f32, tag="pw")
            nc.vector.scalar_tensor_tensor(
                out=pw[:], in0=psO[:], scalar=rd[:, :1],
                in1=bembr_sb[:], op0=mult, op1=add,
            )
            fin = big_p.tile([P, D], f32, tag="fin")
            nc.vector.tensor_tensor(out=fin[:], in0=pw[:], in1=us[:], op=mult)
            nc.scalar.dma_start(out=out_t[w], in_=fin[:])

        for gi in range(len(groups) + LA):
            if gi < len(groups):
                load_stage(gi)
            if gi >= LA:
                body_stage(gi - LA)
                bw, bgw = groups[gi - LA]
                if bgw == GPW - 1:
                    epilogue(bw)

    nc.compile()
    return nc


def _ensure_ntff_hook():
    """Provide the antenv.axon_hooks shim the boot script expects, so
    run_bass_kernel_spmd(trace=True) can capture NTFF profiles under axon."""
    import sys
    import types
    try:
        from antenv.axon_hooks import get_axon_ntff_profile_hook
        if get_axon_ntff_profile_hook() is not None:
            return True
    except ImportError:
        pass
    try:
        import antenv
        mod = types.ModuleType("antenv.axon_hooks")
        mod._hook = None

        def set_axon_ntff_profile_hook(h):
            mod._hook = h

        def get_axon_ntff_profile_hook():
            return mod._hook

        mod.set_axon_ntff_profile_hook = set_axon_ntff_profile_hook
        mod.get_axon_ntff_profile_hook = get_axon_ntff_profile_hook
        sys.modules["antenv.axon_hooks"] = mod
        antenv.axon_hooks = mod
        from trn_agent_boot.trn_boot import _ntff_profile_via_ctypes
        hook = _ntff_profile_via_ctypes("/opt/axon/libaxon_pjrt.so")
        if hook is None:
            return False
        set_axon_ntff_profile_hook(hook)
        return True
    except Exception:
        return False


def _device_run(x, ax, W_emb, b_emb, W_score, b_score, W_size, b_size, idx):
    global LAST_EXEC_NS
    import sys
    for p in ("/opt/trn_rl_repo",):
        if p not in sys.path:
            sys.path.insert(0, p)
    import ml_dtypes
    from concourse.bass_utils import run_bass_kernel_spmd

    bf = ml_dtypes.bfloat16

    counts = np.bincount(idx, minlength=SEG)[:SEG].astype(np.int64)
    order = np.argsort(idx, kind="stable")
    NW = SEG // P                                   # 32 global windows
    wcnt = counts.reshape(NW, P).sum(1)
    TW = max(1, int(-(-wcnt.max() // P)))           # tiles per window

    nc = _BUILD_CACHE.get(TW)
    if nc is None:
        nc = _build_device_program(TW)
        _BUILD_CACHE[TW] = nc

    # Gather map g[c, w, p, t] = node id; pad slots point at row 0 and get a
    # zero one-hot row so they contribute nothing.
    g = np.zeros((NW, TW * P), np.int64)
    valid = np.zeros((NW, TW * P), bool)
    wstart = np.concatenate([[0], np.cumsum(wcnt)])
    for gw in range(NW):
        nodes = order[wstart[gw]:wstart[gw + 1]]
        g[gw, :len(nodes)] = nodes
        valid[gw, :len(nodes)] = True
    # tile-major [TW, P] -> [P, TW]
    g = g.reshape(NCORES, WINS, TW, P).transpose(0, 1, 3, 2)
    valid = valid.reshape(NCORES, WINS, TW, P).transpose(0, 1, 3, 2)

    lseg = (idx[g] - (np.arange(NW).reshape(NCORES, WINS, 1, 1) * P)).astype(np.int64)
    lidx = np.where(valid, lseg, -1).astype(np.float32)
    iota = np.broadcast_to(np.arange(P, dtype=np.float32)[None, :], (P, P)).astype(bf)

    x_bf = x.astype(bf, copy=False)
    ax_bf = ax.astype(bf, copy=False)
    GPW = -(-TW // 4)
    TWp = GPW * 4
    # Pad the gather map to a multiple of 4 tiles; pad tiles point at row 0
    # with lidx=-1 so they contribute nothing.
    gp = np.zeros((NCORES, WINS, P, TWp), np.int64)
    gp[..., :TW] = g
    xg = x_bf[gp]                                   # [NCORES, WINS, P, TWp, D]
    axg = ax_bf[gp]
    # Packed per-group block [c, w, gw, dp/p, slot, 1024]:
    #   [:, k, 0:512]    = attention_x D-chunk k, transposed, node-major
    #   [:, k, 512:1024] = x rows of tile t0+k
    ax_blk = (axg.transpose(0, 1, 3, 2, 4)          # [c, w, t, p, D]
              .reshape(NCORES, WINS, GPW, 4 * P, 4, P)
              .transpose(0, 1, 2, 5, 4, 3))         # [c, w, gw, dp, dc, 512]
    x_blk = (xg.reshape(NCORES, WINS, P, GPW, 4, D)
             .transpose(0, 1, 3, 2, 4, 5))          # [c, w, gw, p, slot, D]
    grp_pack = np.concatenate(
        [np.ascontiguousarray(ax_blk), np.ascontiguousarray(x_blk)], axis=-1)

    wemb = np.ascontiguousarray(
        W_emb.astype(bf).reshape(4, P, D).transpose(1, 0, 2))
    wsc4 = np.ascontiguousarray(W_score.astype(bf)[:, 0].reshape(4, P).T)
    cnts = np.ascontiguousarray(
        counts.astype(np.float32).reshape(NCORES, WINS, P).transpose(0, 2, 1))
    wszr = np.broadcast_to(W_size.astype(np.float32)[0], (P, D)).copy()
    bszr = np.broadcast_to(b_size.astype(np.float32), (P, D)).copy()
    bembr = np.broadcast_to(b_emb.astype(np.float32), (P, D)).copy()
    bscr = np.full((P, 1), np.float32(b_score[0]), np.float32)
    ones = np.ones((P, 1), bf)
    onesf = np.ones((P, 1), np.float32)
    ident = np.eye(P, dtype=bf)

    in_maps = []
    for c in range(NCORES):
        in_maps.append({
            "grp": grp_pack[c],
            "lidx": np.ascontiguousarray(lidx[c]),
            "iota": np.ascontiguousarray(iota),
            "wemb": wemb, "wsc4": wsc4, "cnts": cnts[c],
            "wszr": wszr, "bszr": bszr, "bembr": bembr,
            "bscr": bscr, "ones": ones, "onesf": onesf, "ident": ident,
        })

    trace = bool(os.environ.get("BASS_KERNEL_TRACE"))
    if trace:
        trace = _ensure_ntff_hook()
    res = run_bass_kernel_spmd(nc, in_maps, core_ids=list(range(NCORES)),
                               trace=trace)
    outs = res.results if hasattr(res, "results") else res
    if hasattr(res, "exec_time_ns") and res.exec_time_ns is not None:
        LAST_EXEC_NS = res.exec_time_ns
    out = np.concatenate(
        [np.asarray(o["out"], np.float32).reshape(SEGC, D) for o in outs], axis=0)
    return out


def _host_run(x, ax, W_emb, b_emb, W_score, b_score, W_size, b_size, idx):
    """Numpy fallback (only used if the device toolchain is unavailable)."""
    h = x @ W_emb + b_emb
    scores = (ax @ W_score)[:, 0] + b_score[0]
    order = np.argsort(idx, kind="stable")
    idx_s = idx[order]
    counts = np.bincount(idx_s, minlength=SEG)[:SEG]
    starts = np.zeros(SEG, np.int64)
    np.cumsum(counts[:-1], out=starts[1:])
    starts_c = np.minimum(starts, max(len(idx_s) - 1, 0))
    nonempty = counts > 0
    scores_s = scores[order]
    seg_max = np.maximum.reduceat(scores_s, starts_c)
    wgt = np.exp(scores - seg_max[idx])
    denom = np.add.reduceat(wgt[order], starts_c)
    weights = wgt / (denom[idx] + EPS)
    weighted = (h * weights[:, None]).astype(np.float32)
    pooled = np.add.reduceat(weighted[order], starts_c, axis=0)
    pooled[~nonempty] = 0.0
    updated = counts.astype(np.float32)[:, None] @ W_size + b_size
    return (pooled * updated).astype(np.float32)


def kernel(x, attention_x, W_emb, b_emb, W_score, b_score, W_size, b_size,
           index, size):
    global _DEVICE_OK
    x = np.asarray(x, dtype=np.float32)
    attention_x = np.asarray(attention_x, dtype=np.float32)
    W_emb = np.asarray(W_emb, dtype=np.float32)
    b_emb = np.asarray(b_emb, dtype=np.float32)
    W_score = np.asarray(W_score, dtype=np.float32)
    b_score = np.asarray(b_score, dtype=np.float32)
    W_size = np.asarray(W_size, dtype=np.float32)
    b_size = np.asarray(b_size, dtype=np.float32)
    idx = np.asarray(index).astype(np.int64).ravel()

    try:
        out = _device_run(x, attention_x, W_emb, b_emb, W_score, b_score,
                          W_size, b_size, idx)
        _DEVICE_OK = True
        return out
    except Exception:
        if os.environ.get("BASS_KERNEL_RAISE"):
            raise
        _DEVICE_OK = False
        return _host_run(x, attention_x, W_emb, b_emb, W_score, b_score,
                         W_size, b_size, idx)
